# revision 21
# baseline (speedup 1.0000x reference)
"""Trainium2 Bass kernel for nn_Agent_8778913153394 (gnn_message_passing).

Strategy (8 NeuronCores, data-parallel over the 1024-row batch, 128 rows/core;
full inputs in, full outputs out; all math fp32-exact on device):

  * prev-action embeddings: indirect DMA row gathers (rel_table, ent_table).
  * 2-layer LSTM + 2-layer MLP on TensorE (fp32, activations transposed via
    PE+identity; biases folded into the PSUM accumulation as K=1 matmuls;
    gate nonlinearities on ScalarE straight out of PSUM).
  * candidate entity part of the scores: 256 per-column indirect-DMA gathers
    (HW constraint: one index per partition per call fetching one contiguous
    1KB embedding row) + per-column fused dot on VectorE
    (scalar_tensor_tensor with accum_out).
  * candidate relation part: R = u @ rel_table^T once on TensorE, then a
    per-column fused one-hot select on VectorE:
    relp[:,a] = sum_r (iota_r == nr[:,a]) * R[:,r]  — exact, no extra DMA.
  * masked log-softmax, gumbel-argmax sampling (gumbel noise for the fixed
    key 42 precomputed on host CPU — it is a constant of the problem), loss /
    chosen_relation gathers via iota/equality masks on VectorE.
  * the first candidate gather is made to wait for the LSTM weight DMAs
    (add_dep_helper) so weight traffic is not starved by gather traffic;
    this shortened the measured kernel from ~500us to ~357us.

Host-side work is limited to input sharding / constant preparation (gumbel
noise, transposed copy of the small 400x256 relation table, iota tables,
identity matrix) and output re-assembly. All embedding lookups, the LSTM/MLP,
scoring, softmax and sampling run on device.

Measured (on-device For_i repeat-loop method, 8 cores): ~357 us / invocation,
worst output rel. error ~1.7e-6 vs the jax fp32 reference; action and
chosen_relation exact.
"""

import sys
import numpy as np

for _p in ("/opt/trn_rl_repo",):
    if _p not in sys.path:
        sys.path.insert(0, _p)

B, A, E = 1024, 256, 256          # batch, num actions, embedding dim (per table)
H = 512                           # hidden
RV, EV = 400, 100000              # relation / entity vocab
NCORES = 8
BP = B // NCORES                  # batch rows per core (128)
ACH = 8                           # candidate chunk (columns per gather/dot step)
NEG = -99999.0
DEFER_GATHERS = 16        # gathers wait for this many LSTM weight-tile DMAs (0=off)
REL_VIA_GATHER = False    # True: gather rel rows too; one fused 512-wide dot per column

_BUILT = {}


def _build_program(repeat=1, ablate=()):
    """Build (once) the Bass program shared by all 8 cores."""
    import concourse.bass as bass
    import concourse.mybir as mybir
    import concourse.tile as tile
    from concourse import bacc

    f32 = mybir.dt.float32
    i32 = mybir.dt.int32
    OP = mybir.AluOpType
    AF = mybir.ActivationFunctionType
    AX = mybir.AxisListType

    nc = bacc.Bacc("TRN2", num_devices=NCORES, debug=False)

    # ---------------- DRAM I/O ----------------
    din = {}

    def dt_in(name, shape, dtype=f32):
        din[name] = nc.dram_tensor(name, list(shape), dtype, kind="ExternalInput")
        return din[name]

    dout = {}

    def dt_out(name, shape, dtype=f32):
        dout[name] = nc.dram_tensor(name, list(shape), dtype, kind="ExternalOutput")
        return dout[name]

    # per-core sharded inputs
    dt_in("nr", (BP, A), i32)          # next_relations shard
    dt_in("ne", (BP, A), i32)          # next_entities shard
    dt_in("pr", (BP, 1), i32)          # prev_relation shard
    dt_in("ce", (BP, 1), i32)          # current_entities shard
    dt_in("qe", (BP, E))               # query embedding shard
    dt_in("ph0", (BP, H))
    dt_in("pc0", (BP, H))
    dt_in("ph1", (BP, H))
    dt_in("pc1", (BP, H))
    dt_in("gum", (BP, A))              # gumbel noise shard (fixed key 42)
    # replicated constants
    dt_in("iota_a", (BP, A))           # each row = [0..A)
    dt_in("iota_r", (BP, RV))          # each row = [0..RV)
    dt_in("id128", (128, 128))         # identity for PE transposes
    dt_in("rel", (RV, E))
    dt_in("relT", (E, RV))
    dt_in("ent", (EV, E))
    dt_in("Wx0", (H, 4 * H)); dt_in("Wh0", (H, 4 * H)); dt_in("b0", (1, 4 * H))
    dt_in("Wx1", (H, 4 * H)); dt_in("Wh1", (H, 4 * H)); dt_in("b1", (1, 4 * H))
    dt_in("W1d", (2 * H, H)); dt_in("b1d", (1, H))
    dt_in("W2d", (H, H)); dt_in("b2d", (1, H))

    dt_out("o_loss", (BP, 1))
    dt_out("o_h0", (BP, H)); dt_out("o_c0", (BP, H))
    dt_out("o_h1", (BP, H)); dt_out("o_c1", (BP, H))
    dt_out("o_logp", (BP, A))
    dt_out("o_action", (BP, 1), i32)
    dt_out("o_chosen", (BP, 1), i32)
    dt_out("o_prelim", (BP, A))

    ap = {k: v.ap() for k, v in {**din, **dout}.items()}

    with tile.TileContext(nc) as tc:
        with (
            tc.tile_pool(name="persist", bufs=1) as pp,
            tc.tile_pool(name="wbig", bufs=3) as wb,       # [128,2048] weight stream
            tc.tile_pool(name="wsmall", bufs=4) as ws,     # [128,512] weight stream
            tc.tile_pool(name="cand", bufs=(4 if REL_VIA_GATHER else 5)) as cp,       # gathered cand chunks
            tc.tile_pool(name="zpsum", bufs=4, space="PSUM") as zp,
            tc.tile_pool(name="tpsum", bufs=2, space="PSUM") as tp,
        ):
          import contextlib
          _loop_cm = tc.For_i(0, repeat, 1) if repeat > 1 else contextlib.nullcontext()
          with _loop_cm:
            # ---------- small input loads ----------
            def load(name, shape, dtype=f32):
                t = pp.tile(list(shape), dtype, tag=name)
                nc.sync.dma_start(out=t[:], in_=ap[name])
                return t

            ne_sb = load("ne", (BP, A), i32)
            nr_sb = load("nr", (BP, A), i32)
            pr_sb = load("pr", (BP, 1), i32)
            ce_sb = load("ce", (BP, 1), i32)
            gum_sb = load("gum", (BP, A))
            iota_sb = load("iota_a", (BP, A))
            iotar_sb = load("iota_r", (BP, RV))
            id_sb = load("id128", (128, 128))
            qe_sb = load("qe", (BP, E))
            ph0_sb = load("ph0", (BP, H)); pc0_sb = load("pc0", (BP, H))
            ph1_sb = load("ph1", (BP, H)); pc1_sb = load("pc1", (BP, H))
            b0_sb = load("b0", (1, 4 * H)); b1_sb = load("b1", (1, 4 * H))
            b1d_sb = load("b1d", (1, H)); b2d_sb = load("b2d", (1, H))  # noqa
            relT_sb = pp.tile([128, 2, RV], f32, tag="relT")  # relT k-chunks
            for k in range(2):
                nc.sync.dma_start(out=relT_sb[:, k, :],
                                  in_=ap["relT"][k * 128:(k + 1) * 128, :])

            ones1 = pp.tile([1, 128], f32, tag="ones1")
            nc.vector.memset(ones1[:], 1.0)

            # ---------- candidate entity-embedding gather (big, starts early) ----
            IOA = bass.IndirectOffsetOnAxis
            nchunks = A // ACH
            cand_tiles = []
            gather_insts = []
            CW = 2 * E if REL_VIA_GATHER else E   # candidate row width
            for c in range(nchunks):
                ct = cp.tile([BP, ACH, CW], f32, tag="cand")
                for j in range(ACH):
                    a = c * ACH + j
                    if REL_VIA_GATHER:
                        gi0 = nc.gpsimd.indirect_dma_start(
                            out=ct[:, j, 0:E], out_offset=None, in_=ap["rel"],
                            in_offset=IOA(ap=nr_sb[:, a:a + 1], axis=0))
                        gather_insts.append(gi0)
                    gi_inst = nc.gpsimd.indirect_dma_start(
                        out=ct[:, j, CW - E:CW],
                        out_offset=None,
                        in_=ap["ent"],
                        in_offset=IOA(ap=ne_sb[:, a:a + 1], axis=0),
                    )
                    gather_insts.append(gi_inst)
                cand_tiles.append(ct)

            # ---------- prev action embeddings ----------
            prev_act = pp.tile([BP, 2 * E], f32, tag="prev_act")
            nc.gpsimd.indirect_dma_start(
                out=prev_act[:, 0:E], out_offset=None, in_=ap["rel"],
                in_offset=IOA(ap=pr_sb[:, 0:1], axis=0),
            )
            nc.gpsimd.indirect_dma_start(
                out=prev_act[:, E:2 * E], out_offset=None, in_=ap["ent"],
                in_offset=IOA(ap=ce_sb[:, 0:1], axis=0),
            )

            # ---------- helpers ----------
            def transpose_to(dst, src, k):
                """dst[:, k, :] = src[:, 128k:128(k+1)].T  (via PE + ACT copy)"""
                t = tp.tile([128, 128], f32, tag="tp")
                nc.tensor.transpose(out=t[:], in_=src[:, k * 128:(k + 1) * 128],
                                    identity=id_sb[:])
                nc.scalar.copy(out=dst[:, k, :], in_=t[:])

            def matmulT(x_sb, nk):
                """Return SBUF tile [128, nk, 128] = x_sb^T in k-chunks."""
                xt = pp.tile([128, nk, 128], f32, tag=f"xt{nk}_{matmulT.i % 2}", name=f"xt_{matmulT.i}")
                matmulT.i += 1
                for k in range(nk):
                    transpose_to(xt, x_sb, k)
                return xt
            matmulT.i = 0

            weight_insts = []

            def lstm_layer(x_sb, h_sb, c_sb, wx_name, wh_name, b_sb, oh, oc):
                xT = matmulT(x_sb, 4)
                hT = matmulT(h_sb, 4)
                zt = [zp.tile([128, H], f32, tag="z", name=f"zt{lstm_layer.i}_{_n}") for _n in range(4)]
                for src, wname in ((xT, wx_name), (hT, wh_name)):
                    for k in range(4):
                        w = wb.tile([128, 4 * H], f32, tag="w")
                        wi = nc.sync.dma_start(out=w[:],
                                          in_=ap[wname][k * 128:(k + 1) * 128, :])
                        weight_insts.append(wi)
                        for n in range(4):
                            nc.tensor.matmul(
                                out=zt[n][:], lhsT=src[:, k, :],
                                rhs=w[:, n * H:(n + 1) * H],
                                start=(src is xT and k == 0), stop=False)
                for n in range(4):
                    nc.tensor.matmul(out=zt[n][:], lhsT=ones1[:],
                                     rhs=b_sb[:, n * H:(n + 1) * H],
                                     start=False, stop=True)
                gi = pp.tile([128, H], f32, tag="gi")
                gf = pp.tile([128, H], f32, tag="gf")
                gg = pp.tile([128, H], f32, tag="gg")
                go = pp.tile([128, H], f32, tag="go")
                nc.scalar.activation(out=gi[:], in_=zt[0][:], func=AF.Sigmoid)
                nc.scalar.activation(out=gf[:], in_=zt[1][:], func=AF.Sigmoid)
                nc.scalar.activation(out=gg[:], in_=zt[2][:], func=AF.Tanh)
                nc.scalar.activation(out=go[:], in_=zt[3][:], func=AF.Sigmoid)
                t1 = pp.tile([128, H], f32, tag="t1")
                t2 = pp.tile([128, H], f32, tag="t2")
                cn = pp.tile([128, H], f32, tag=f"cn{lstm_layer.i}")
                hn = pp.tile([128, H], f32, tag=f"hn{lstm_layer.i}")
                nc.vector.tensor_tensor(out=t1[:], in0=gf[:], in1=c_sb[:], op=OP.mult)
                nc.vector.tensor_tensor(out=t2[:], in0=gi[:], in1=gg[:], op=OP.mult)
                nc.vector.tensor_tensor(out=cn[:], in0=t1[:], in1=t2[:], op=OP.add)
                nc.scalar.activation(out=t1[:], in_=cn[:], func=AF.Tanh)
                nc.vector.tensor_tensor(out=hn[:], in0=go[:], in1=t1[:], op=OP.mult)
                nc.sync.dma_start(out=oh, in_=hn[:])
                nc.sync.dma_start(out=oc, in_=cn[:])
                lstm_layer.i += 1
                return hn, cn
            lstm_layer.i = 0

            h0, c0 = lstm_layer(prev_act, ph0_sb, pc0_sb, "Wx0", "Wh0", b0_sb,
                                ap["o_h0"], ap["o_c0"])
            h1, c1 = lstm_layer(h0, ph1_sb, pc1_sb, "Wx1", "Wh1", b1_sb,
                                ap["o_h1"], ap["o_c1"])

            # Deprioritize gather DMA traffic: first gather waits until the
            # layer-0 weight tiles have landed (DEFER_GATHERS of them).
            if DEFER_GATHERS and gather_insts and weight_insts:
                from concourse.tile_rust import add_dep_helper
                add_dep_helper(gather_insts[0].ins,
                               weight_insts[min(DEFER_GATHERS, len(weight_insts)) - 1].ins,
                               sync=True, reason="defer gathers behind weight DMAs")

            # ---------- MLP ----------
            sq = pp.tile([BP, 2 * H], f32, tag="sq")     # [h1 | prev_ent | qe]
            nc.scalar.copy(out=sq[:, 0:H], in_=h1[:])
            nc.scalar.copy(out=sq[:, H:H + E], in_=prev_act[:, E:2 * E])
            nc.scalar.copy(out=sq[:, H + E:2 * H], in_=qe_sb[:])

            sqT = matmulT(sq, 8)
            hid_ps = zp.tile([128, H], f32, tag="z")
            for k in range(8):
                w = ws.tile([128, H], f32, tag="wsm")
                nc.sync.dma_start(out=w[:], in_=ap["W1d"][k * 128:(k + 1) * 128, :])
                nc.tensor.matmul(out=hid_ps[:], lhsT=sqT[:, k, :], rhs=w[:],
                                 start=(k == 0), stop=False)
            nc.tensor.matmul(out=hid_ps[:], lhsT=ones1[:], rhs=b1d_sb[:],
                             start=False, stop=True)
            hidden = pp.tile([128, H], f32, tag="hidden")
            nc.scalar.activation(out=hidden[:], in_=hid_ps[:], func=AF.Relu)

            hidT = matmulT(hidden, 4)
            mlp_ps = zp.tile([128, H], f32, tag="z")
            for k in range(4):
                w = ws.tile([128, H], f32, tag="wsm")
                nc.sync.dma_start(out=w[:], in_=ap["W2d"][k * 128:(k + 1) * 128, :])
                nc.tensor.matmul(out=mlp_ps[:], lhsT=hidT[:, k, :], rhs=w[:],
                                 start=(k == 0), stop=False)
            nc.tensor.matmul(out=mlp_ps[:], lhsT=ones1[:], rhs=b2d_sb[:],
                             start=False, stop=True)
            mlp = pp.tile([128, H], f32, tag="mlp")
            nc.scalar.activation(out=mlp[:], in_=mlp_ps[:], func=AF.Relu)

            prelim = pp.tile([BP, A], f32, tag="prelim")
            if REL_VIA_GATHER:
                # one fused 512-wide dot per column: prelim = <[rel|ent] row, mlp>
                scr = pp.tile([BP, 2 * E], f32, tag="scr")
                for c in range(nchunks):
                    ct = cand_tiles[c]
                    for j in range(ACH):
                        a = c * ACH + j
                        nc.vector.scalar_tensor_tensor(
                            out=scr[:], in0=ct[:, j, :], scalar=1.0, in1=mlp[:],
                            op0=OP.mult, op1=OP.mult, accum_out=prelim[:, a:a + 1])
            else:
                # R = u @ relT on PE, then exact one-hot select per column
                uT = matmulT(mlp, 2)                    # mlp[:, :256]^T (2 chunks)
                R_ps = zp.tile([128, RV], f32, tag="z")
                for k in range(2):
                    nc.tensor.matmul(out=R_ps[:], lhsT=uT[:, k, :],
                                     rhs=relT_sb[:, k, :],
                                     start=(k == 0), stop=(k == 1))
                R_sb = pp.tile([128, RV], f32, tag="R")
                nc.vector.tensor_copy(out=R_sb[:], in_=R_ps[:])
                nr_f32 = pp.tile([BP, A], f32, tag="nr_f32")
                nc.scalar.copy(out=nr_f32[:], in_=nr_sb[:])
                relp = pp.tile([BP, A], f32, tag="relp")
                rscr = pp.tile([BP, RV], f32, tag="rscr")
                for a in range(A):
                    # relp[:, a] = sum_r (iota_r == nr[:, a]) * R[:, r]
                    nc.vector.scalar_tensor_tensor(
                        out=rscr[:], in0=iotar_sb[:], scalar=nr_f32[:, a:a + 1],
                        in1=R_sb[:], op0=OP.is_equal, op1=OP.mult,
                        accum_out=relp[:, a:a + 1])
                v_ap = mlp[:, E:2 * E]                   # [128, 256]
                entp = pp.tile([BP, A], f32, tag="entp")
                scr = pp.tile([BP, E], f32, tag="scr")
                for c in range(nchunks):
                    ct = cand_tiles[c]
                    for j in range(ACH):
                        a = c * ACH + j
                        nc.vector.scalar_tensor_tensor(
                            out=scr[:], in0=ct[:, j, :], scalar=1.0, in1=v_ap,
                            op0=OP.mult, op1=OP.mult, accum_out=entp[:, a:a + 1])
                nc.vector.tensor_tensor(out=prelim[:], in0=entp[:], in1=relp[:], op=OP.add)
            nc.sync.dma_start(out=ap["o_prelim"], in_=prelim[:])

            mask = pp.tile([BP, A], i32, tag="mask")
            nc.vector.tensor_scalar(out=mask[:], in0=nr_sb[:], scalar1=0.0,
                                    scalar2=None, op0=OP.is_equal)
            negt = pp.tile([BP, A], f32, tag="negt")
            nc.vector.memset(negt[:], NEG)
            scores = pp.tile([BP, A], f32, tag="scores")
            nc.vector.select(out=scores[:], mask=mask[:], on_true=negt[:],
                             on_false=prelim[:])

            m = pp.tile([BP, 1], f32, tag="m")
            nc.vector.reduce_max(out=m[:], in_=scores[:], axis=AX.X)
            neg_m = pp.tile([BP, 1], f32, tag="neg_m")
            nc.vector.tensor_scalar(out=neg_m[:], in0=m[:], scalar1=-1.0,
                                    scalar2=None, op0=OP.mult)
            esum = pp.tile([BP, 1], f32, tag="esum")
            et = pp.tile([BP, A], f32, tag="et")
            nc.scalar.activation(out=et[:], in_=scores[:], func=AF.Exp,
                                 bias=neg_m[:, 0:1], accum_out=esum[:])
            lns = pp.tile([BP, 1], f32, tag="lns")
            nc.scalar.activation(out=lns[:], in_=esum[:], func=AF.Ln)
            logp = pp.tile([BP, A], f32, tag="logp")
            nc.vector.tensor_scalar(out=logp[:], in0=scores[:], scalar1=m[:, 0:1],
                                    scalar2=lns[:, 0:1], op0=OP.subtract,
                                    op1=OP.subtract)
            nc.sync.dma_start(out=ap["o_logp"], in_=logp[:])

            # action = argmax(scores + gumbel)  (first index wins ties)
            xg = pp.tile([BP, A], f32, tag="xg")
            nc.vector.tensor_tensor(out=xg[:], in0=scores[:], in1=gum_sb[:], op=OP.add)
            mx = pp.tile([BP, 1], f32, tag="mx")
            nc.vector.reduce_max(out=mx[:], in_=xg[:], axis=AX.X)
            eq = pp.tile([BP, A], i32, tag="eq")
            nc.vector.tensor_scalar(out=eq[:], in0=xg[:], scalar1=mx[:, 0:1],
                                    scalar2=None, op0=OP.is_equal)
            bigt = pp.tile([BP, A], f32, tag="bigt")
            nc.vector.memset(bigt[:], 1e9)
            selidx = pp.tile([BP, A], f32, tag="selidx")
            nc.vector.select(out=selidx[:], mask=eq[:], on_true=iota_sb[:],
                             on_false=bigt[:])
            act_f = pp.tile([BP, 1], f32, tag="act_f")
            nc.vector.tensor_reduce(out=act_f[:], in_=selidx[:], axis=AX.X,
                                    op=OP.min)
            act_i = pp.tile([BP, 1], i32, tag="act_i")
            nc.vector.tensor_copy(out=act_i[:], in_=act_f[:])
            nc.sync.dma_start(out=ap["o_action"], in_=act_i[:])

            # loss = -logp[b, action]; chosen = nr[b, action]
            eq2 = pp.tile([BP, A], f32, tag="eq2")
            nc.vector.tensor_scalar(out=eq2[:], in0=iota_sb[:],
                                    scalar1=act_f[:, 0:1], scalar2=None,
                                    op0=OP.is_equal)
            lsel = pp.tile([BP, A], f32, tag="lsel")
            nc.vector.tensor_tensor(out=lsel[:], in0=eq2[:], in1=logp[:], op=OP.mult)
            loss = pp.tile([BP, 1], f32, tag="loss")
            nc.vector.reduce_sum(out=loss[:], in_=lsel[:], axis=AX.X)
            nloss = pp.tile([BP, 1], f32, tag="nloss")
            nc.vector.tensor_scalar(out=nloss[:], in0=loss[:], scalar1=-1.0,
                                    scalar2=None, op0=OP.mult)
            nc.sync.dma_start(out=ap["o_loss"], in_=nloss[:])

            nr_f = pp.tile([BP, A], f32, tag="nr_f")
            nc.scalar.copy(out=nr_f[:], in_=nr_sb[:])
            csel = pp.tile([BP, A], f32, tag="csel")
            nc.vector.tensor_tensor(out=csel[:], in0=eq2[:], in1=nr_f[:], op=OP.mult)
            cho_f = pp.tile([BP, 1], f32, tag="cho_f")
            nc.vector.reduce_sum(out=cho_f[:], in_=csel[:], axis=AX.X)
            cho_i = pp.tile([BP, 1], i32, tag="cho_i")
            nc.vector.tensor_copy(out=cho_i[:], in_=cho_f[:])
            nc.sync.dma_start(out=ap["o_chosen"], in_=cho_i[:])

    nc.compile()
    return nc


def _get_program(repeat=1):
    key = f"nc{repeat}"
    if key not in _BUILT:
        _BUILT[key] = _build_program(repeat)
    return _BUILT[key]


def _gumbel():
    if "gum" not in _BUILT:
        import jax
        import jax.numpy as jnp
        cpu = jax.local_devices(backend="cpu")[0]
        with jax.default_device(cpu):
            g = jax.random.gumbel(jax.random.key(42), (B, A), jnp.float32)
            _BUILT["gum"] = np.asarray(g)
    return _BUILT["gum"]


def _host_prep(inputs):
    """Build the 8 per-core input maps."""
    f = {k: np.asarray(v) for k, v in inputs.items()}
    gum = _gumbel()
    shared = {
        "iota_a": np.tile(np.arange(A, dtype=np.float32), (BP, 1)),
        "iota_r": np.tile(np.arange(RV, dtype=np.float32), (BP, 1)),
        "id128": np.eye(128, dtype=np.float32),
        "rel": f["rel_table"].astype(np.float32),
        "relT": np.ascontiguousarray(f["rel_table"].T.astype(np.float32)),
        "ent": f["ent_table"].astype(np.float32),
        "Wx0": f["Wx0"], "Wh0": f["Wh0"], "b0": f["b0"].reshape(1, -1),
        "Wx1": f["Wx1"], "Wh1": f["Wh1"], "b1": f["b1"].reshape(1, -1),
        "W1d": f["W1d"], "b1d": f["b1d"].reshape(1, -1),
        "W2d": f["W2d"], "b2d": f["b2d"].reshape(1, -1),
    }
    shared = {k: np.ascontiguousarray(v) for k, v in shared.items()}
    in_maps = []
    rows = np.arange(BP, dtype=np.int64)
    for k in range(NCORES):
        s = slice(k * BP, (k + 1) * BP)
        nr = f["next_relations"][s].astype(np.int32)
        m = {
            "nr": nr,
            "ne": f["next_entities"][s].astype(np.int32),
            "pr": f["prev_relation"][s].astype(np.int32).reshape(BP, 1),
            "ce": f["current_entities"][s].astype(np.int32).reshape(BP, 1),
            "qe": f["query_embedding"][s],
            "ph0": f["prev_h0"][s], "pc0": f["prev_c0"][s],
            "ph1": f["prev_h1"][s], "pc1": f["prev_c1"][s],
            "gum": gum[s],
        }
        m = {k2: np.ascontiguousarray(v) for k2, v in m.items()}
        m.update(shared)
        in_maps.append(m)
    return in_maps


def run(inputs, trace=False):
    from concourse.bass_utils import run_bass_kernel_spmd

    nc = _get_program()
    in_maps = _host_prep(inputs)
    res = run_bass_kernel_spmd(nc, in_maps, core_ids=list(range(NCORES)),
                               trace=trace)
    r = res.results
    loss = np.concatenate([r[k]["o_loss"][:, 0] for k in range(NCORES)])
    h0 = np.concatenate([r[k]["o_h0"] for k in range(NCORES)])
    c0 = np.concatenate([r[k]["o_c0"] for k in range(NCORES)])
    h1 = np.concatenate([r[k]["o_h1"] for k in range(NCORES)])
    c1 = np.concatenate([r[k]["o_c1"] for k in range(NCORES)])
    logp = np.concatenate([r[k]["o_logp"] for k in range(NCORES)])
    action = np.concatenate([r[k]["o_action"][:, 0] for k in range(NCORES)])
    chosen = np.concatenate([r[k]["o_chosen"][:, 0] for k in range(NCORES)])
    prelim = np.concatenate([r[k]["o_prelim"] for k in range(NCORES)])
    out = (loss, h0, c0, h1, c1, logp, action.astype(np.int32),
           chosen.astype(np.int32), prelim)
    return out, res


def kernel(**inputs):
    out, _ = run(inputs, trace=False)
    return out


# revision 22
# speedup vs baseline: 1.5027x; 1.5027x over previous
"""Trainium2 Bass kernel for nn_Agent_8778913153394 (gnn_message_passing).

Strategy (8 NeuronCores, data-parallel over the 1024-row batch, 128 rows/core;
full inputs in, full outputs out; all math fp32-exact on device):

  * prev-action embeddings: indirect DMA row gathers (rel_table, ent_table).
  * 2-layer LSTM + 2-layer MLP on TensorE (fp32, activations transposed via
    PE+identity; biases folded into the PSUM accumulation as K=1 matmuls;
    gate nonlinearities on ScalarE straight out of PSUM).
  * candidate entity part of the scores: 256 per-column indirect-DMA gathers
    (HW constraint: one index per partition per call fetching one contiguous
    1KB embedding row) + per-column fused dot on VectorE
    (scalar_tensor_tensor with accum_out).
  * candidate relation part: R = u @ rel_table^T once on TensorE, then a
    per-column fused one-hot select on VectorE:
    relp[:,a] = sum_r (iota_r == nr[:,a]) * R[:,r]  — exact, no extra DMA.
  * masked log-softmax, gumbel-argmax sampling (gumbel noise for the fixed
    key 42 precomputed on host CPU — it is a constant of the problem), loss /
    chosen_relation gathers via iota/equality masks on VectorE.
  * the first candidate gather is made to wait for the LSTM weight DMAs
    (add_dep_helper) so weight traffic is not starved by gather traffic;
    this shortened the measured kernel from ~500us to ~357us.

Host-side work is limited to input sharding / constant preparation (gumbel
noise, transposed copy of the small 400x256 relation table, iota tables,
identity matrix) and output re-assembly. All embedding lookups, the LSTM/MLP,
scoring, softmax and sampling run on device.

Measured (on-device For_i repeat-loop method, 8 cores): ~355 us per the
production cost model, ~360-470 us across wall-clock samples (the axon
round-trip noise of +-60 us dominates the estimator); worst output rel.
error ~1.7e-6 vs the jax fp32 reference; action and chosen_relation exact.
"""

import sys
import numpy as np

for _p in ("/opt/trn_rl_repo",):
    if _p not in sys.path:
        sys.path.insert(0, _p)

B, A, E = 1024, 256, 256          # batch, num actions, embedding dim (per table)
H = 512                           # hidden
RV, EV = 400, 100000              # relation / entity vocab
NCORES = 8
BP = B // NCORES                  # batch rows per core (128)
ACH = 8                           # candidate chunk (columns per gather/dot step)
NEG = -99999.0
DEFER_GATHERS = 16        # gathers wait for this many LSTM weight-tile DMAs (0=off)
REL_VIA_GATHER = False    # True: gather rel rows too; one fused 512-wide dot per column

_BUILT = {}


def _build_program(repeat=1, ablate=()):
    """Build (once) the Bass program shared by all 8 cores."""
    import concourse.bass as bass
    import concourse.mybir as mybir
    import concourse.tile as tile
    from concourse import bacc

    f32 = mybir.dt.float32
    i32 = mybir.dt.int32
    OP = mybir.AluOpType
    AF = mybir.ActivationFunctionType
    AX = mybir.AxisListType

    nc = bacc.Bacc("TRN2", num_devices=NCORES, debug=False)

    # ---------------- DRAM I/O ----------------
    din = {}

    def dt_in(name, shape, dtype=f32):
        din[name] = nc.dram_tensor(name, list(shape), dtype, kind="ExternalInput")
        return din[name]

    dout = {}

    def dt_out(name, shape, dtype=f32):
        dout[name] = nc.dram_tensor(name, list(shape), dtype, kind="ExternalOutput")
        return dout[name]

    # per-core sharded inputs
    dt_in("nr", (BP, A), i32)          # next_relations shard
    dt_in("ne", (BP, A), i32)          # next_entities shard
    dt_in("pr", (BP, 1), i32)          # prev_relation shard
    dt_in("ce", (BP, 1), i32)          # current_entities shard
    dt_in("qe", (BP, E))               # query embedding shard
    dt_in("ph0", (BP, H))
    dt_in("pc0", (BP, H))
    dt_in("ph1", (BP, H))
    dt_in("pc1", (BP, H))
    dt_in("gum", (BP, A))              # gumbel noise shard (fixed key 42)
    # replicated constants
    dt_in("iota_a", (BP, A))           # each row = [0..A)
    dt_in("iota_r", (BP, RV))          # each row = [0..RV)
    dt_in("id128", (128, 128))         # identity for PE transposes
    dt_in("rel", (RV, E))
    dt_in("relT", (E, RV))
    dt_in("ent", (EV, E))
    dt_in("Wx0", (H, 4 * H)); dt_in("Wh0", (H, 4 * H)); dt_in("b0", (1, 4 * H))
    dt_in("Wx1", (H, 4 * H)); dt_in("Wh1", (H, 4 * H)); dt_in("b1", (1, 4 * H))
    dt_in("W1d", (2 * H, H)); dt_in("b1d", (1, H))
    dt_in("W2d", (H, H)); dt_in("b2d", (1, H))

    dt_out("o_loss", (BP, 1))
    dt_out("o_h0", (BP, H)); dt_out("o_c0", (BP, H))
    dt_out("o_h1", (BP, H)); dt_out("o_c1", (BP, H))
    dt_out("o_logp", (BP, A))
    dt_out("o_action", (BP, 1), i32)
    dt_out("o_chosen", (BP, 1), i32)
    dt_out("o_prelim", (BP, A))

    ap = {k: v.ap() for k, v in {**din, **dout}.items()}

    with tile.TileContext(nc) as tc:
        with (
            tc.tile_pool(name="persist", bufs=1) as pp,
            tc.tile_pool(name="wbig", bufs=3) as wb,       # [128,2048] weight stream
            tc.tile_pool(name="wsmall", bufs=4) as ws,     # [128,512] weight stream
            tc.tile_pool(name="cand", bufs=(4 if REL_VIA_GATHER else 5)) as cp,       # gathered cand chunks
            tc.tile_pool(name="zpsum", bufs=4, space="PSUM") as zp,
            tc.tile_pool(name="tpsum", bufs=2, space="PSUM") as tp,
        ):
          import contextlib
          _loop_cm = tc.For_i(0, repeat, 1) if repeat > 1 else contextlib.nullcontext()
          with _loop_cm:
            # ---------- small input loads ----------
            def load(name, shape, dtype=f32):
                t = pp.tile(list(shape), dtype, tag=name)
                nc.sync.dma_start(out=t[:], in_=ap[name])
                return t

            ne_sb = load("ne", (BP, A), i32)
            nr_sb = load("nr", (BP, A), i32)
            pr_sb = load("pr", (BP, 1), i32)
            ce_sb = load("ce", (BP, 1), i32)
            gum_sb = load("gum", (BP, A))
            iota_sb = load("iota_a", (BP, A))
            iotar_sb = load("iota_r", (BP, RV))
            id_sb = load("id128", (128, 128))
            qe_sb = load("qe", (BP, E))
            ph0_sb = load("ph0", (BP, H)); pc0_sb = load("pc0", (BP, H))
            ph1_sb = load("ph1", (BP, H)); pc1_sb = load("pc1", (BP, H))
            b0_sb = load("b0", (1, 4 * H)); b1_sb = load("b1", (1, 4 * H))
            b1d_sb = load("b1d", (1, H)); b2d_sb = load("b2d", (1, H))  # noqa
            relT_sb = pp.tile([128, 2, RV], f32, tag="relT")  # relT k-chunks
            for k in range(2):
                nc.sync.dma_start(out=relT_sb[:, k, :],
                                  in_=ap["relT"][k * 128:(k + 1) * 128, :])

            ones1 = pp.tile([1, 128], f32, tag="ones1")
            nc.vector.memset(ones1[:], 1.0)

            # ---------- candidate entity-embedding gather (big, starts early) ----
            IOA = bass.IndirectOffsetOnAxis
            nchunks = A // ACH
            cand_tiles = []
            gather_insts = []
            CW = 2 * E if REL_VIA_GATHER else E   # candidate row width
            for c in range(nchunks):
                ct = cp.tile([BP, ACH, CW], f32, tag="cand")
                for j in range(ACH):
                    a = c * ACH + j
                    if REL_VIA_GATHER:
                        gi0 = nc.gpsimd.indirect_dma_start(
                            out=ct[:, j, 0:E], out_offset=None, in_=ap["rel"],
                            in_offset=IOA(ap=nr_sb[:, a:a + 1], axis=0))
                        gather_insts.append(gi0)
                    gi_inst = nc.gpsimd.indirect_dma_start(
                        out=ct[:, j, CW - E:CW],
                        out_offset=None,
                        in_=ap["ent"],
                        in_offset=IOA(ap=ne_sb[:, a:a + 1], axis=0),
                    )
                    gather_insts.append(gi_inst)
                cand_tiles.append(ct)

            # ---------- prev action embeddings ----------
            prev_act = pp.tile([BP, 2 * E], f32, tag="prev_act")
            nc.gpsimd.indirect_dma_start(
                out=prev_act[:, 0:E], out_offset=None, in_=ap["rel"],
                in_offset=IOA(ap=pr_sb[:, 0:1], axis=0),
            )
            nc.gpsimd.indirect_dma_start(
                out=prev_act[:, E:2 * E], out_offset=None, in_=ap["ent"],
                in_offset=IOA(ap=ce_sb[:, 0:1], axis=0),
            )

            # ---------- helpers ----------
            def transpose_to(dst, src, k):
                """dst[:, k, :] = src[:, 128k:128(k+1)].T  (via PE + ACT copy)"""
                t = tp.tile([128, 128], f32, tag="tp")
                nc.tensor.transpose(out=t[:], in_=src[:, k * 128:(k + 1) * 128],
                                    identity=id_sb[:])
                nc.scalar.copy(out=dst[:, k, :], in_=t[:])

            def matmulT(x_sb, nk):
                """Return SBUF tile [128, nk, 128] = x_sb^T in k-chunks."""
                xt = pp.tile([128, nk, 128], f32, tag=f"xt{nk}_{matmulT.i % 2}", name=f"xt_{matmulT.i}")
                matmulT.i += 1
                for k in range(nk):
                    transpose_to(xt, x_sb, k)
                return xt
            matmulT.i = 0

            weight_insts = []

            def lstm_layer(x_sb, h_sb, c_sb, wx_name, wh_name, b_sb, oh, oc):
                xT = matmulT(x_sb, 4)
                hT = matmulT(h_sb, 4)
                zt = [zp.tile([128, H], f32, tag="z", name=f"zt{lstm_layer.i}_{_n}") for _n in range(4)]
                for src, wname in ((xT, wx_name), (hT, wh_name)):
                    for k in range(4):
                        w = wb.tile([128, 4 * H], f32, tag="w")
                        wi = nc.sync.dma_start(out=w[:],
                                          in_=ap[wname][k * 128:(k + 1) * 128, :])
                        weight_insts.append(wi)
                        for n in range(4):
                            nc.tensor.matmul(
                                out=zt[n][:], lhsT=src[:, k, :],
                                rhs=w[:, n * H:(n + 1) * H],
                                start=(src is xT and k == 0), stop=False)
                for n in range(4):
                    nc.tensor.matmul(out=zt[n][:], lhsT=ones1[:],
                                     rhs=b_sb[:, n * H:(n + 1) * H],
                                     start=False, stop=True)
                gi = pp.tile([128, H], f32, tag="gi")
                gf = pp.tile([128, H], f32, tag="gf")
                gg = pp.tile([128, H], f32, tag="gg")
                go = pp.tile([128, H], f32, tag="go")
                nc.scalar.activation(out=gi[:], in_=zt[0][:], func=AF.Sigmoid)
                nc.scalar.activation(out=gf[:], in_=zt[1][:], func=AF.Sigmoid)
                nc.scalar.activation(out=gg[:], in_=zt[2][:], func=AF.Tanh)
                nc.scalar.activation(out=go[:], in_=zt[3][:], func=AF.Sigmoid)
                t1 = pp.tile([128, H], f32, tag="t1")
                t2 = pp.tile([128, H], f32, tag="t2")
                cn = pp.tile([128, H], f32, tag=f"cn{lstm_layer.i}")
                hn = pp.tile([128, H], f32, tag=f"hn{lstm_layer.i}")
                nc.vector.tensor_tensor(out=t1[:], in0=gf[:], in1=c_sb[:], op=OP.mult)
                nc.vector.tensor_tensor(out=t2[:], in0=gi[:], in1=gg[:], op=OP.mult)
                nc.vector.tensor_tensor(out=cn[:], in0=t1[:], in1=t2[:], op=OP.add)
                nc.scalar.activation(out=t1[:], in_=cn[:], func=AF.Tanh)
                nc.vector.tensor_tensor(out=hn[:], in0=go[:], in1=t1[:], op=OP.mult)
                nc.sync.dma_start(out=oh, in_=hn[:])
                nc.sync.dma_start(out=oc, in_=cn[:])
                lstm_layer.i += 1
                return hn, cn
            lstm_layer.i = 0

            h0, c0 = lstm_layer(prev_act, ph0_sb, pc0_sb, "Wx0", "Wh0", b0_sb,
                                ap["o_h0"], ap["o_c0"])
            h1, c1 = lstm_layer(h0, ph1_sb, pc1_sb, "Wx1", "Wh1", b1_sb,
                                ap["o_h1"], ap["o_c1"])

            # Deprioritize gather DMA traffic: first gather waits until the
            # layer-0 weight tiles have landed (DEFER_GATHERS of them).
            if DEFER_GATHERS and gather_insts and weight_insts:
                from concourse.tile_rust import add_dep_helper
                add_dep_helper(gather_insts[0].ins,
                               weight_insts[min(DEFER_GATHERS, len(weight_insts)) - 1].ins,
                               sync=True, reason="defer gathers behind weight DMAs")

            # ---------- MLP ----------
            sq = pp.tile([BP, 2 * H], f32, tag="sq")     # [h1 | prev_ent | qe]
            nc.scalar.copy(out=sq[:, 0:H], in_=h1[:])
            nc.scalar.copy(out=sq[:, H:H + E], in_=prev_act[:, E:2 * E])
            nc.scalar.copy(out=sq[:, H + E:2 * H], in_=qe_sb[:])

            sqT = matmulT(sq, 8)
            hid_ps = zp.tile([128, H], f32, tag="z")
            for k in range(8):
                w = ws.tile([128, H], f32, tag="wsm")
                nc.sync.dma_start(out=w[:], in_=ap["W1d"][k * 128:(k + 1) * 128, :])
                nc.tensor.matmul(out=hid_ps[:], lhsT=sqT[:, k, :], rhs=w[:],
                                 start=(k == 0), stop=False)
            nc.tensor.matmul(out=hid_ps[:], lhsT=ones1[:], rhs=b1d_sb[:],
                             start=False, stop=True)
            hidden = pp.tile([128, H], f32, tag="hidden")
            nc.scalar.activation(out=hidden[:], in_=hid_ps[:], func=AF.Relu)

            hidT = matmulT(hidden, 4)
            mlp_ps = zp.tile([128, H], f32, tag="z")
            for k in range(4):
                w = ws.tile([128, H], f32, tag="wsm")
                nc.sync.dma_start(out=w[:], in_=ap["W2d"][k * 128:(k + 1) * 128, :])
                nc.tensor.matmul(out=mlp_ps[:], lhsT=hidT[:, k, :], rhs=w[:],
                                 start=(k == 0), stop=False)
            nc.tensor.matmul(out=mlp_ps[:], lhsT=ones1[:], rhs=b2d_sb[:],
                             start=False, stop=True)
            mlp = pp.tile([128, H], f32, tag="mlp")
            nc.scalar.activation(out=mlp[:], in_=mlp_ps[:], func=AF.Relu)

            prelim = pp.tile([BP, A], f32, tag="prelim")
            if REL_VIA_GATHER:
                # one fused 512-wide dot per column: prelim = <[rel|ent] row, mlp>
                scr = pp.tile([BP, 2 * E], f32, tag="scr")
                for c in range(nchunks):
                    ct = cand_tiles[c]
                    for j in range(ACH):
                        a = c * ACH + j
                        nc.vector.scalar_tensor_tensor(
                            out=scr[:], in0=ct[:, j, :], scalar=1.0, in1=mlp[:],
                            op0=OP.mult, op1=OP.mult, accum_out=prelim[:, a:a + 1])
            else:
                # R = u @ relT on PE, then exact one-hot select per column
                uT = matmulT(mlp, 2)                    # mlp[:, :256]^T (2 chunks)
                R_ps = zp.tile([128, RV], f32, tag="z")
                for k in range(2):
                    nc.tensor.matmul(out=R_ps[:], lhsT=uT[:, k, :],
                                     rhs=relT_sb[:, k, :],
                                     start=(k == 0), stop=(k == 1))
                R_sb = pp.tile([128, RV], f32, tag="R")
                nc.vector.tensor_copy(out=R_sb[:], in_=R_ps[:])
                nr_f32 = pp.tile([BP, A], f32, tag="nr_f32")
                nc.scalar.copy(out=nr_f32[:], in_=nr_sb[:])
                relp = pp.tile([BP, A], f32, tag="relp")
                rscr = pp.tile([BP, RV], f32, tag="rscr")
                for a in range(A):
                    # relp[:, a] = sum_r (iota_r == nr[:, a]) * R[:, r]
                    nc.vector.scalar_tensor_tensor(
                        out=rscr[:], in0=iotar_sb[:], scalar=nr_f32[:, a:a + 1],
                        in1=R_sb[:], op0=OP.is_equal, op1=OP.mult,
                        accum_out=relp[:, a:a + 1])
                v_ap = mlp[:, E:2 * E]                   # [128, 256]
                entp = pp.tile([BP, A], f32, tag="entp")
                scr = pp.tile([BP, E], f32, tag="scr")
                for c in range(nchunks):
                    ct = cand_tiles[c]
                    for j in range(ACH):
                        a = c * ACH + j
                        nc.vector.scalar_tensor_tensor(
                            out=scr[:], in0=ct[:, j, :], scalar=1.0, in1=v_ap,
                            op0=OP.mult, op1=OP.mult, accum_out=entp[:, a:a + 1])
                nc.vector.tensor_tensor(out=prelim[:], in0=entp[:], in1=relp[:], op=OP.add)
            nc.sync.dma_start(out=ap["o_prelim"], in_=prelim[:])

            mask = pp.tile([BP, A], i32, tag="mask")
            nc.vector.tensor_scalar(out=mask[:], in0=nr_sb[:], scalar1=0.0,
                                    scalar2=None, op0=OP.is_equal)
            negt = pp.tile([BP, A], f32, tag="negt")
            nc.vector.memset(negt[:], NEG)
            scores = pp.tile([BP, A], f32, tag="scores")
            nc.vector.select(out=scores[:], mask=mask[:], on_true=negt[:],
                             on_false=prelim[:])

            m = pp.tile([BP, 1], f32, tag="m")
            nc.vector.reduce_max(out=m[:], in_=scores[:], axis=AX.X)
            neg_m = pp.tile([BP, 1], f32, tag="neg_m")
            nc.vector.tensor_scalar(out=neg_m[:], in0=m[:], scalar1=-1.0,
                                    scalar2=None, op0=OP.mult)
            esum = pp.tile([BP, 1], f32, tag="esum")
            et = pp.tile([BP, A], f32, tag="et")
            nc.scalar.activation(out=et[:], in_=scores[:], func=AF.Exp,
                                 bias=neg_m[:, 0:1], accum_out=esum[:])
            lns = pp.tile([BP, 1], f32, tag="lns")
            nc.scalar.activation(out=lns[:], in_=esum[:], func=AF.Ln)
            logp = pp.tile([BP, A], f32, tag="logp")
            nc.vector.tensor_scalar(out=logp[:], in0=scores[:], scalar1=m[:, 0:1],
                                    scalar2=lns[:, 0:1], op0=OP.subtract,
                                    op1=OP.subtract)
            nc.sync.dma_start(out=ap["o_logp"], in_=logp[:])

            # action = argmax(scores + gumbel)  (first index wins ties)
            xg = pp.tile([BP, A], f32, tag="xg")
            nc.vector.tensor_tensor(out=xg[:], in0=scores[:], in1=gum_sb[:], op=OP.add)
            mx = pp.tile([BP, 1], f32, tag="mx")
            nc.vector.reduce_max(out=mx[:], in_=xg[:], axis=AX.X)
            eq = pp.tile([BP, A], i32, tag="eq")
            nc.vector.tensor_scalar(out=eq[:], in0=xg[:], scalar1=mx[:, 0:1],
                                    scalar2=None, op0=OP.is_equal)
            bigt = pp.tile([BP, A], f32, tag="bigt")
            nc.vector.memset(bigt[:], 1e9)
            selidx = pp.tile([BP, A], f32, tag="selidx")
            nc.vector.select(out=selidx[:], mask=eq[:], on_true=iota_sb[:],
                             on_false=bigt[:])
            act_f = pp.tile([BP, 1], f32, tag="act_f")
            nc.vector.tensor_reduce(out=act_f[:], in_=selidx[:], axis=AX.X,
                                    op=OP.min)
            act_i = pp.tile([BP, 1], i32, tag="act_i")
            nc.vector.tensor_copy(out=act_i[:], in_=act_f[:])
            nc.sync.dma_start(out=ap["o_action"], in_=act_i[:])

            # loss = -logp[b, action]; chosen = nr[b, action]
            eq2 = pp.tile([BP, A], f32, tag="eq2")
            nc.vector.tensor_scalar(out=eq2[:], in0=iota_sb[:],
                                    scalar1=act_f[:, 0:1], scalar2=None,
                                    op0=OP.is_equal)
            lsel = pp.tile([BP, A], f32, tag="lsel")
            nc.vector.tensor_tensor(out=lsel[:], in0=eq2[:], in1=logp[:], op=OP.mult)
            loss = pp.tile([BP, 1], f32, tag="loss")
            nc.vector.reduce_sum(out=loss[:], in_=lsel[:], axis=AX.X)
            nloss = pp.tile([BP, 1], f32, tag="nloss")
            nc.vector.tensor_scalar(out=nloss[:], in0=loss[:], scalar1=-1.0,
                                    scalar2=None, op0=OP.mult)
            nc.sync.dma_start(out=ap["o_loss"], in_=nloss[:])

            nr_f = pp.tile([BP, A], f32, tag="nr_f")
            nc.scalar.copy(out=nr_f[:], in_=nr_sb[:])
            csel = pp.tile([BP, A], f32, tag="csel")
            nc.vector.tensor_tensor(out=csel[:], in0=eq2[:], in1=nr_f[:], op=OP.mult)
            cho_f = pp.tile([BP, 1], f32, tag="cho_f")
            nc.vector.reduce_sum(out=cho_f[:], in_=csel[:], axis=AX.X)
            cho_i = pp.tile([BP, 1], i32, tag="cho_i")
            nc.vector.tensor_copy(out=cho_i[:], in_=cho_f[:])
            nc.sync.dma_start(out=ap["o_chosen"], in_=cho_i[:])

    nc.compile()
    return nc


def _get_program(repeat=1):
    key = f"nc{repeat}"
    if key not in _BUILT:
        _BUILT[key] = _build_program(repeat)
    return _BUILT[key]


def _gumbel():
    if "gum" not in _BUILT:
        import jax
        import jax.numpy as jnp
        cpu = jax.local_devices(backend="cpu")[0]
        with jax.default_device(cpu):
            g = jax.random.gumbel(jax.random.key(42), (B, A), jnp.float32)
            _BUILT["gum"] = np.asarray(g)
    return _BUILT["gum"]


def _host_prep(inputs):
    """Build the 8 per-core input maps."""
    f = {k: np.asarray(v) for k, v in inputs.items()}
    gum = _gumbel()
    shared = {
        "iota_a": np.tile(np.arange(A, dtype=np.float32), (BP, 1)),
        "iota_r": np.tile(np.arange(RV, dtype=np.float32), (BP, 1)),
        "id128": np.eye(128, dtype=np.float32),
        "rel": f["rel_table"].astype(np.float32),
        "relT": np.ascontiguousarray(f["rel_table"].T.astype(np.float32)),
        "ent": f["ent_table"].astype(np.float32),
        "Wx0": f["Wx0"], "Wh0": f["Wh0"], "b0": f["b0"].reshape(1, -1),
        "Wx1": f["Wx1"], "Wh1": f["Wh1"], "b1": f["b1"].reshape(1, -1),
        "W1d": f["W1d"], "b1d": f["b1d"].reshape(1, -1),
        "W2d": f["W2d"], "b2d": f["b2d"].reshape(1, -1),
    }
    shared = {k: np.ascontiguousarray(v) for k, v in shared.items()}
    in_maps = []
    rows = np.arange(BP, dtype=np.int64)
    for k in range(NCORES):
        s = slice(k * BP, (k + 1) * BP)
        nr = f["next_relations"][s].astype(np.int32)
        m = {
            "nr": nr,
            "ne": f["next_entities"][s].astype(np.int32),
            "pr": f["prev_relation"][s].astype(np.int32).reshape(BP, 1),
            "ce": f["current_entities"][s].astype(np.int32).reshape(BP, 1),
            "qe": f["query_embedding"][s],
            "ph0": f["prev_h0"][s], "pc0": f["prev_c0"][s],
            "ph1": f["prev_h1"][s], "pc1": f["prev_c1"][s],
            "gum": gum[s],
        }
        m = {k2: np.ascontiguousarray(v) for k2, v in m.items()}
        m.update(shared)
        in_maps.append(m)
    return in_maps


def run(inputs, trace=False):
    from concourse.bass_utils import run_bass_kernel_spmd

    nc = _get_program()
    in_maps = _host_prep(inputs)
    res = run_bass_kernel_spmd(nc, in_maps, core_ids=list(range(NCORES)),
                               trace=trace)
    r = res.results
    loss = np.concatenate([r[k]["o_loss"][:, 0] for k in range(NCORES)])
    h0 = np.concatenate([r[k]["o_h0"] for k in range(NCORES)])
    c0 = np.concatenate([r[k]["o_c0"] for k in range(NCORES)])
    h1 = np.concatenate([r[k]["o_h1"] for k in range(NCORES)])
    c1 = np.concatenate([r[k]["o_c1"] for k in range(NCORES)])
    logp = np.concatenate([r[k]["o_logp"] for k in range(NCORES)])
    action = np.concatenate([r[k]["o_action"][:, 0] for k in range(NCORES)])
    chosen = np.concatenate([r[k]["o_chosen"][:, 0] for k in range(NCORES)])
    prelim = np.concatenate([r[k]["o_prelim"] for k in range(NCORES)])
    out = (loss, h0, c0, h1, c1, logp, action.astype(np.int32),
           chosen.astype(np.int32), prelim)
    return out, res


def kernel(**inputs):
    out, _ = run(inputs, trace=False)
    return out
